# revision 8
# baseline (speedup 1.0000x reference)
"""Trainium2 Bass kernel: batched polynomial + Fourier-series point evaluator.

Math: for each point n and each of B=4 times t_b:
    y_poly[b, n]    = sum_{i<4}  poly[n, i] * t_b^i
    y_fourier[b, n] = sum_{k<18} fa[n, k]*cos(w_k t_b) + fb[n, k]*sin(w_k t_b)
(with Fourier bands gated by model_stage).

Because B=4 is tiny, both outputs are one linear map applied to the 40
per-point coefficients:  Y[:, n] = Basis.T @ W[n, :]  with Basis [40, 8]
computed on host (the transcendentals depend only on the 4 scalar times).
The device kernel is a pure streaming matmul over the coefficient tables.

Layout per core (points sharded 8 ways, NC = 2^18 points/core):
  - host packs coefficients as fp16 [80, NC/2]: two "point groups" (halves
    of the core's range) stacked along the contraction dim, so each matmul
    column carries 2 points (80 coeffs).
  - 8 matmuls with block-diagonal stationary tiles [80, 128] accumulate
    into one [128, 512] PSUM bank; row 16*s + 8*g + j holds output j
    (0-3 poly batch, 4-7 fourier batch) of group g for sub-block s.
    One bank fill = 8192 points, copied PSUM->SBUF densely by DVE.
  - output DMAs de-interleave straight into the [4, NC] fp32 outputs.
"""

import json

import numpy as np

import concourse.bass as bass
import concourse.mybir as mybir
import concourse.tile as tile
from concourse.bass_utils import run_bass_kernel_spmd

# Problem constants (hardcoded per harness contract).
B = 4
N_POINTS = 128 ** 3            # 2097152
N_CORES = 8
NC = N_POINTS // N_CORES       # 262144 points per core
H = NC // 2                    # 131072 points per group (half)
KH = 18                        # harmonics
NCOEF = 40                     # 4 poly + 18 cos + 18 sin

WT = 4096                      # in-tile free width (points per group per tile)
N_TILES = H // WT              # 32 in-tiles per core
U = 8                          # in-tiles per output span
SPANS = N_TILES // U           # 4
MM_N = 512                     # matmul moving free size (one PSUM bank of fp32)
S_BLK = 8                      # matmuls per bank fill (WT / MM_N)

_CACHED_NC = None
LAST_RESULTS = None            # BassKernelResults of the most recent run


def _build_module():
    nc = bass.Bass()
    dt = mybir.dt

    table = nc.dram_tensor("table", [80, H], dt.float16, kind="ExternalInput")
    basis = nc.dram_tensor("basis", [80, 128 * S_BLK], dt.float16, kind="ExternalInput")
    out_p = nc.dram_tensor("out_poly", [B, NC], dt.float32, kind="ExternalOutput")
    out_f = nc.dram_tensor("out_four", [B, NC], dt.float32, kind="ExternalOutput")

    with tile.TileContext(nc) as tc:
        with (
            tc.tile_pool(name="const", bufs=1) as cpool,
            tc.tile_pool(name="inp", bufs=8) as ipool,
            tc.tile_pool(name="psum", bufs=8, space="PSUM") as ppool,
            tc.tile_pool(name="outp", bufs=4) as opool,
        ):
            basis_sb = cpool.tile([80, 128 * S_BLK], dt.float16)
            nc.gpsimd.dma_start(basis_sb[:, :], basis[:, :])

            for span in range(SPANS):
                out_tile = opool.tile([128, U * MM_N], dt.float32)
                for u in range(U):
                    w = span * U + u
                    in_tile = ipool.tile([80, WT], dt.float16)
                    nc.gpsimd.dma_start(
                        in_tile[:, :], table[:, w * WT : (w + 1) * WT]
                    )
                    ps = ppool.tile([128, MM_N], dt.float32)
                    for s in range(S_BLK):
                        # Block s of the stationary basis is nonzero only in
                        # columns [16s, 16s+16); the other rows accumulate +0.
                        nc.tensor.matmul(
                            ps[:, :],
                            basis_sb[:, 128 * s : 128 * (s + 1)],
                            in_tile[:, MM_N * s : MM_N * (s + 1)],
                            start=(s == 0),
                            stop=(s == S_BLK - 1),
                        )
                    nc.vector.tensor_copy(
                        out_tile[:, MM_N * u : MM_N * (u + 1)], ps[:, :]
                    )
                # With the host-side s<->u point permutation, row 16s+8g+j of
                # out_tile holds output j of group g for the contiguous point
                # run [span*U*WT + s*WT, +WT) of half g.
                for s in range(S_BLK):
                    for g in range(2):
                        row0 = 16 * s + 8 * g
                        dst_p = out_p.rearrange(
                            "b (g sp s uf) -> sp s g b uf",
                            g=2, sp=SPANS, s=S_BLK, uf=U * MM_N,
                        )[span, s, g]
                        dst_f = out_f.rearrange(
                            "b (g sp s uf) -> sp s g b uf",
                            g=2, sp=SPANS, s=S_BLK, uf=U * MM_N,
                        )[span, s, g]
                        nc.gpsimd.dma_start(dst_p, out_tile[row0 : row0 + 4, :])
                        nc.gpsimd.dma_start(dst_f, out_tile[row0 + 4 : row0 + 8, :])
    return nc


def _legalize_single_wait(bir_bytes: bytes) -> bytes:
    """Split multi-wait instructions: this walrus build's codegen accepts at
    most ONE sync-wait per ISA instruction.  Hoist all but the last wait onto
    NoOps inserted just before the instruction on the same engine stream
    (the sequencer executes them in order, so semantics are preserved)."""
    m = json.loads(bir_bytes)
    n_split = 0

    def fix_block(b):
        nonlocal n_split
        out = []
        for ins in b.get("instructions", []):
            si = ins.get("sync_info")
            waits = (si or {}).get("on_wait", [])
            if len(waits) > 1 and ins.get("engine", "Unassigned") != "Unassigned":
                for w in waits[:-1]:
                    n_split += 1
                    out.append({
                        "debug": ins.get("debug", 0),
                        "engine": ins["engine"],
                        "ins": [],
                        "name": f"{ins['name']}-wsplit{n_split}",
                        "opcode": "NoOp",
                        "outs": [],
                        "sync_info": {"on_update": [], "on_wait": [w]},
                    })
                si["on_wait"] = [waits[-1]]
            out.append(ins)
        b["instructions"] = out
        for ch in b.get("blocks", []):
            fix_block(ch)

    for fn in m["functions"]:
        for b in fn.get("blocks", []):
            fix_block(b)
    return json.dumps(m).encode()


def _get_module():
    global _CACHED_NC
    if _CACHED_NC is None:
        nc = _build_module()
        orig = nc.to_json_bytes
        nc.to_json_bytes = lambda: _legalize_single_wait(orig())
        _CACHED_NC = nc
    return _CACHED_NC


def _host_basis(input_t: np.ndarray, model_stage) -> np.ndarray:
    """Build the packed stationary weights [80, 1024] fp16."""
    stage = int(model_stage)
    curr = min(stage, 3) if stage >= 0 else 3
    mask = np.zeros(KH, dtype=np.float64)
    for s, e, req in ((0, 3, 1), (3, 9, 2), (9, KH, 3)):
        if curr >= req:
            mask[s:e] = 1.0

    t = np.asarray(input_t, dtype=np.float64)
    Vp = np.stack([t ** i for i in range(4)], axis=0)          # [4, B]
    w = 2.0 * np.pi * np.arange(1, KH + 1, dtype=np.float64)   # [18]
    C = np.cos(np.outer(w, t)) * mask[:, None]                 # [18, B]
    S = np.sin(np.outer(w, t)) * mask[:, None]                 # [18, B]

    B8 = np.zeros((NCOEF, 8), dtype=np.float64)
    B8[0:4, 0:4] = Vp
    B8[4:22, 4:8] = C
    B8[22:40, 4:8] = S
    B8 = B8.astype(np.float16)

    basis_all = np.zeros((80, 128 * S_BLK), dtype=np.float16)
    for s in range(S_BLK):
        for g in range(2):
            col0 = 128 * s + 16 * s + 8 * g
            basis_all[40 * g : 40 * g + 40, col0 : col0 + 8] = B8
    return basis_all


def kernel(input_t, poly_coeffs, fourier_a, fourier_b, model_stage):
    global LAST_RESULTS
    input_t = np.asarray(input_t, dtype=np.float32)
    poly_coeffs = np.asarray(poly_coeffs, dtype=np.float32)
    fourier_a = np.asarray(fourier_a, dtype=np.float32)
    fourier_b = np.asarray(fourier_b, dtype=np.float32)
    assert input_t.shape == (B,)
    assert poly_coeffs.shape == (N_POINTS, 4)
    assert fourier_a.shape == (N_POINTS, KH)
    assert fourier_b.shape == (N_POINTS, KH)

    basis_all = _host_basis(input_t, model_stage)

    # Pack per-core coefficient tables: [80, H] fp16, rows 40g+k = coeff k
    # of point-half g.  The point axis is additionally permuted so the
    # device's (u, s) iteration order lands output rows on contiguous DRAM
    # runs: table column span*U*WT + u*WT + s*MM_N + f holds point
    # span*U*WT + s*WT + u*MM_N + f of the half.
    W = np.concatenate([poly_coeffs, fourier_a, fourier_b], axis=1)
    W = W.astype(np.float16)                                   # [N, 40]
    Wr = W.reshape(N_CORES, 2, H, NCOEF).transpose(0, 1, 3, 2)  # [8, 2, 40, H]
    Wr = Wr.reshape(N_CORES, 2, NCOEF, SPANS, S_BLK, U, MM_N)
    Wr = Wr.transpose(0, 1, 2, 3, 5, 4, 6)                     # swap s <-> u
    Wr = np.ascontiguousarray(Wr).reshape(N_CORES, 80, H)

    nc = _get_module()
    in_maps = [
        {"table": Wr[c], "basis": basis_all} for c in range(N_CORES)
    ]
    LAST_RESULTS = run_bass_kernel_spmd(nc, in_maps, core_ids=list(range(N_CORES)))
    results = LAST_RESULTS.results

    y_poly = np.concatenate([r["out_poly"] for r in results], axis=1)
    y_fourier = np.concatenate([r["out_four"] for r in results], axis=1)
    return y_poly, y_fourier


# revision 9
# speedup vs baseline: 1.6782x; 1.6782x over previous
"""Trainium2 Bass kernel: batched polynomial + Fourier-series point evaluator.

Math: for each point n and each of B=4 times t_b:
    y_poly[b, n]    = sum_{i<4}  poly[n, i] * t_b^i
    y_fourier[b, n] = sum_{k<18} fa[n, k]*cos(w_k t_b) + fb[n, k]*sin(w_k t_b)
(with Fourier bands gated by model_stage).

Because B=4 is tiny, both outputs are one linear map applied to the 40
per-point coefficients:  Y[:, n] = Basis.T @ W[n, :]  with Basis [40, 8]
computed on host (the transcendentals depend only on the 4 scalar times).
The device kernel is a pure streaming matmul over the coefficient tables.

Per-core layout (points sharded 8 ways, ~2^18 points/core, padded to
NP = 3*C so the contraction dim packs GROUPS=3 point-groups of 40 coeffs
= K=120):
  - host packs coefficients as fp16 [120, C]; each matmul column carries
    3 points, so one N=512 matmul evaluates 1536 points.
  - 4 matmuls per PSUM bank at tile_position (0, 32j) run concurrently on
    disjoint 32-column strips of the PE array (same [120, 32] stationary
    basis), filling a [128, 512] bank with 6144 points' outputs.
  - PSUM -> SBUF copies cast to fp16; row 32j+8g+jj then holds output jj
    (0-3 poly batch, 4-7 fourier batch) of group g on a contiguous point
    run, so output DMAs are plain 2D slices.
"""

import json

import numpy as np

import concourse.bass as bass
import concourse.mybir as mybir
import concourse.tile as tile
from concourse.bass_utils import run_bass_kernel_spmd

# Problem constants (hardcoded per harness contract).
B = 4
N_POINTS = 128 ** 3            # 2097152
N_CORES = 8
NC = N_POINTS // N_CORES       # 262144 real points per core
KH = 18                        # harmonics
NCOEF = 40                     # 4 poly + 18 cos + 18 sin

GROUPS = 3                     # point-groups stacked in contraction dim (K=120)
JT = 4                         # concurrent col-strip matmuls per PSUM bank
MM_N = 512                     # matmul moving free size (one PSUM bank of fp32)
SPANS = 4
U = 11                         # PSUM bank fills per span
BANK_COLS = JT * MM_N          # 2048 table columns per bank fill
C = SPANS * U * BANK_COLS      # 90112 table columns per core
NP = GROUPS * C                # 270336 padded points per core

_CACHED_NC = None
LAST_RESULTS = None            # BassKernelResults of the most recent run


def _build_module():
    nc = bass.Bass()
    dt = mybir.dt

    table = nc.dram_tensor("table", [GROUPS * NCOEF, C], dt.float16,
                           kind="ExternalInput")
    basis = nc.dram_tensor("basis", [GROUPS * NCOEF, 32], dt.float16,
                           kind="ExternalInput")
    out8 = nc.dram_tensor("out8", [8, NP], dt.float16, kind="ExternalOutput")

    with tile.TileContext(nc) as tc:
        with (
            tc.tile_pool(name="const", bufs=1) as cpool,
            tc.tile_pool(name="inp", bufs=8) as ipool,
            tc.tile_pool(name="psum", bufs=8, space="PSUM") as ppool,
            tc.tile_pool(name="outp", bufs=3) as opool,
        ):
            basis_sb = cpool.tile([GROUPS * NCOEF, 32], dt.float16)
            nc.sync.dma_start(basis_sb[:, :], basis[:, :])

            for span in range(SPANS):
                out_tile = opool.tile([128, U * MM_N], dt.float16)
                for u in range(U):
                    q = span * U + u
                    in_tile = ipool.tile([GROUPS * NCOEF, BANK_COLS], dt.float16)
                    nc.sync.dma_start(
                        in_tile[:, :], table[:, q * BANK_COLS : (q + 1) * BANK_COLS]
                    )
                    ps = ppool.tile([128, MM_N], dt.float32)
                    for j in range(JT):
                        nc.tensor.matmul(
                            ps[32 * j : 32 * (j + 1), :],
                            basis_sb[:, :],
                            in_tile[:, MM_N * j : MM_N * (j + 1)],
                            start=True,
                            stop=True,
                            tile_position=(0, 32 * j),
                        )
                    nc.vector.tensor_copy(
                        out_tile[:, MM_N * u : MM_N * (u + 1)], ps[:, :]
                    )
                # Row 32j+8g+jj holds output jj of group g for the contiguous
                # point run starting at g*C + span*U*BANK_COLS + j*U*MM_N.
                for j in range(JT):
                    for g in range(GROUPS):
                        row0 = 32 * j + 8 * g
                        col0 = g * C + span * U * BANK_COLS + j * U * MM_N
                        nc.scalar.dma_start(
                            out8[0:8, col0 : col0 + U * MM_N],
                            out_tile[row0 : row0 + 8, :],
                        )
    return nc


def _legalize_single_wait(bir_bytes: bytes) -> bytes:
    """Split multi-wait instructions: this walrus build's codegen accepts at
    most ONE sync-wait per ISA instruction.  Hoist all but the last wait onto
    NoOps inserted just before the instruction on the same engine stream
    (the sequencer executes them in order, so semantics are preserved)."""
    m = json.loads(bir_bytes)
    n_split = 0

    def fix_block(b):
        nonlocal n_split
        out = []
        for ins in b.get("instructions", []):
            si = ins.get("sync_info")
            waits = (si or {}).get("on_wait", [])
            if len(waits) > 1 and ins.get("engine", "Unassigned") != "Unassigned":
                for w in waits[:-1]:
                    n_split += 1
                    out.append({
                        "debug": ins.get("debug", 0),
                        "engine": ins["engine"],
                        "ins": [],
                        "name": f"{ins['name']}-wsplit{n_split}",
                        "opcode": "NoOp",
                        "outs": [],
                        "sync_info": {"on_update": [], "on_wait": [w]},
                    })
                si["on_wait"] = [waits[-1]]
            out.append(ins)
        b["instructions"] = out
        for ch in b.get("blocks", []):
            fix_block(ch)

    for fn in m["functions"]:
        for b in fn.get("blocks", []):
            fix_block(b)
    return json.dumps(m).encode()


def _get_module():
    global _CACHED_NC
    if _CACHED_NC is None:
        nc = _build_module()
        orig = nc.to_json_bytes
        nc.to_json_bytes = lambda: _legalize_single_wait(orig())
        _CACHED_NC = nc
    return _CACHED_NC


def _host_basis(input_t: np.ndarray, model_stage) -> np.ndarray:
    """Packed stationary weights [120, 32] fp16: col 8g+jj = output jj of
    point-group g (jj 0-3 poly batch, 4-7 fourier batch)."""
    stage = int(model_stage)
    curr = min(stage, 3) if stage >= 0 else 3
    mask = np.zeros(KH, dtype=np.float64)
    for s, e, req in ((0, 3, 1), (3, 9, 2), (9, KH, 3)):
        if curr >= req:
            mask[s:e] = 1.0

    t = np.asarray(input_t, dtype=np.float64)
    Vp = np.stack([t ** i for i in range(4)], axis=0)           # [4, B]
    w = 2.0 * np.pi * np.arange(1, KH + 1, dtype=np.float64)    # [18]
    Cc = np.cos(np.outer(w, t)) * mask[:, None]                 # [18, B]
    Ss = np.sin(np.outer(w, t)) * mask[:, None]                 # [18, B]

    B8 = np.zeros((NCOEF, 8), dtype=np.float64)
    B8[0:4, 0:4] = Vp
    B8[4:22, 4:8] = Cc
    B8[22:40, 4:8] = Ss

    basis = np.zeros((GROUPS * NCOEF, 32), dtype=np.float64)
    for g in range(GROUPS):
        basis[NCOEF * g : NCOEF * (g + 1), 8 * g : 8 * g + 8] = B8
    return basis.astype(np.float16)


def kernel(input_t, poly_coeffs, fourier_a, fourier_b, model_stage):
    global LAST_RESULTS
    input_t = np.asarray(input_t, dtype=np.float32)
    poly_coeffs = np.asarray(poly_coeffs, dtype=np.float32)
    fourier_a = np.asarray(fourier_a, dtype=np.float32)
    fourier_b = np.asarray(fourier_b, dtype=np.float32)
    assert input_t.shape == (B,)
    assert poly_coeffs.shape == (N_POINTS, 4)
    assert fourier_a.shape == (N_POINTS, KH)
    assert fourier_b.shape == (N_POINTS, KH)

    basis = _host_basis(input_t, model_stage)

    # Pack per-core tables [120, C] fp16 with the device's column order:
    # table col (span*U + u)*BANK_COLS + j*MM_N + f  holds point
    # g*C + span*U*BANK_COLS + j*U*MM_N + u*MM_N + f   (j <-> u swapped so
    # each output row covers a contiguous DRAM run).
    W = np.concatenate([poly_coeffs, fourier_a, fourier_b], axis=1)
    W = W.astype(np.float16)                                    # [N, 40]
    Wp = np.zeros((N_CORES, NP, NCOEF), dtype=np.float16)
    Wp[:, :NC] = W.reshape(N_CORES, NC, NCOEF)
    Wp = Wp.reshape(N_CORES, GROUPS, SPANS, JT, U, MM_N, NCOEF)
    Wp = Wp.transpose(0, 1, 6, 2, 4, 3, 5)   # core, g, k, span, u, j, f
    tables = np.ascontiguousarray(Wp).reshape(N_CORES, GROUPS * NCOEF, C)

    nc = _get_module()
    in_maps = [{"table": tables[c], "basis": basis} for c in range(N_CORES)]
    LAST_RESULTS = run_bass_kernel_spmd(nc, in_maps, core_ids=list(range(N_CORES)))
    results = LAST_RESULTS.results

    out = np.concatenate(
        [r["out8"][:, :NC].astype(np.float32) for r in results], axis=1
    )
    return out[0:4], out[4:8]


# revision 11
# speedup vs baseline: 1.7542x; 1.0453x over previous
"""Trainium2 Bass kernel: batched polynomial + Fourier-series point evaluator.

Math: for each point n and each of B=4 times t_b:
    y_poly[b, n]    = sum_{i<4}  poly[n, i] * t_b^i
    y_fourier[b, n] = sum_{k<18} fa[n, k]*cos(w_k t_b) + fb[n, k]*sin(w_k t_b)
(with Fourier bands gated by model_stage).

Because B=4 is tiny, both outputs are one linear map applied to the 40
per-point coefficients:  Y[:, n] = Basis.T @ W[n, :]  with Basis [40, 8]
computed on host (the transcendentals depend only on the 4 scalar times).
The device kernel is a pure streaming matmul over the coefficient tables.

Per-core layout (points sharded 8 ways, ~2^18 points/core, padded to
NP = 3*C so the contraction dim packs GROUPS=3 point-groups of 40 coeffs
= K=120):
  - host packs coefficients as fp16 [120, C]; each matmul column carries
    3 points, so one N=512 matmul evaluates 1536 points.
  - 4 matmuls per PSUM bank at tile_position (0, 32j) run concurrently on
    disjoint 32-column strips of the PE array (same [120, 32] stationary
    basis), filling a [128, 512] bank with 6144 points' outputs.
  - PSUM -> SBUF copies cast to fp16; row 32j+8g+jj then holds output jj
    (0-3 poly batch, 4-7 fourier batch) of group g on a contiguous point
    run, so output DMAs are plain 2D slices.
"""

import json

import numpy as np

import concourse.bass as bass
import concourse.mybir as mybir
import concourse.tile as tile
from concourse.bass_utils import run_bass_kernel_spmd

# Problem constants (hardcoded per harness contract).
B = 4
N_POINTS = 128 ** 3            # 2097152
N_CORES = 8
NC = N_POINTS // N_CORES       # 262144 real points per core
KH = 18                        # harmonics
NCOEF = 40                     # 4 poly + 18 cos + 18 sin

GROUPS = 3                     # point-groups stacked in contraction dim (K=120)
JT = 4                         # concurrent col-strip matmuls per PSUM bank
MM_N = 512                     # matmul moving free size (one PSUM bank of fp32)
SPANS = 4
U = 11                         # PSUM bank fills per span
BANK_COLS = JT * MM_N          # 2048 table columns per bank fill
C = SPANS * U * BANK_COLS      # 90112 table columns per core
NP = GROUPS * C                # 270336 padded points per core

_CACHED_NC = None
LAST_RESULTS = None            # BassKernelResults of the most recent run


def _build_module():
    nc = bass.Bass()
    dt = mybir.dt

    table = nc.dram_tensor("table", [GROUPS * NCOEF, C], dt.float16,
                           kind="ExternalInput")
    basis = nc.dram_tensor("basis", [GROUPS * NCOEF, 32], dt.float16,
                           kind="ExternalInput")
    out8 = nc.dram_tensor("out8", [8, NP], dt.float16, kind="ExternalOutput")

    with tile.TileContext(nc) as tc:
        with (
            tc.tile_pool(name="const", bufs=1) as cpool,
            tc.tile_pool(name="inp", bufs=8) as ipool,
            tc.tile_pool(name="psum", bufs=8, space="PSUM") as ppool,
            tc.tile_pool(name="outp", bufs=3) as opool,
        ):
            basis_sb = cpool.tile([GROUPS * NCOEF, 32], dt.float16)
            nc.sync.dma_start(basis_sb[:, :], basis[:, :])

            for span in range(SPANS):
                out_tile = opool.tile([128, U * MM_N], dt.float16)
                for u in range(U):
                    q = span * U + u
                    in_tile = ipool.tile([GROUPS * NCOEF, BANK_COLS], dt.float16)
                    nc.sync.dma_start(
                        in_tile[:, :], table[:, q * BANK_COLS : (q + 1) * BANK_COLS]
                    )
                    ps = ppool.tile([128, MM_N], dt.float32)
                    for j in range(JT):
                        nc.tensor.matmul(
                            ps[32 * j : 32 * (j + 1), :],
                            basis_sb[:, :],
                            in_tile[:, MM_N * j : MM_N * (j + 1)],
                            start=True,
                            stop=True,
                            tile_position=(0, 32 * j),
                        )
                    nc.vector.tensor_copy(
                        out_tile[:, MM_N * u : MM_N * (u + 1)], ps[:, :]
                    )
                # Row 32j+8g+jj holds output jj of group g for the contiguous
                # point run starting at g*C + span*U*BANK_COLS + j*U*MM_N.
                for j in range(JT):
                    for g in range(GROUPS):
                        row0 = 32 * j + 8 * g
                        col0 = g * C + span * U * BANK_COLS + j * U * MM_N
                        nc.scalar.dma_start(
                            out8[0:8, col0 : col0 + U * MM_N],
                            out_tile[row0 : row0 + 8, :],
                        )
    return nc


def _dedupe_ldweights(m: dict) -> None:
    """Drop Ldweights instructions that reload the exact same stationary
    operand into the same PE array position as the previously retained one
    (the weights are static in this kernel).  Any waits on a dropped
    Ldweights migrate to the next instruction in the same engine stream."""
    def sig(ins):
        return json.dumps(
            {k: ins.get(k) for k in ("ins", "tile_position", "perf_mode",
                                     "is_transpose", "tile_size")},
            sort_keys=True,
        )

    def fix_block(b):
        last_by_pos = {}
        out = []
        pending_waits = []
        for ins in b.get("instructions", []):
            if ins.get("opcode") == "Ldweights":
                pos = tuple(ins.get("tile_position") or (0, 0))
                s = sig(ins)
                upd = (ins.get("sync_info") or {}).get("on_update", [])
                if last_by_pos.get(pos) == s and not upd:
                    pending_waits.extend(
                        (ins.get("sync_info") or {}).get("on_wait", []))
                    continue
                last_by_pos[pos] = s
            elif pending_waits and ins.get("engine") == "PE":
                si = ins.setdefault("sync_info", {"on_update": [], "on_wait": []})
                si["on_wait"] = pending_waits + si.get("on_wait", [])
                pending_waits = []
            out.append(ins)
        assert not pending_waits
        b["instructions"] = out
        for ch in b.get("blocks", []):
            fix_block(ch)

    for fn in m["functions"]:
        for b in fn.get("blocks", []):
            fix_block(b)


def _legalize_single_wait(bir_bytes: bytes) -> bytes:
    """Split multi-wait instructions: this walrus build's codegen accepts at
    most ONE sync-wait per ISA instruction.  Hoist all but the last wait onto
    NoOps inserted just before the instruction on the same engine stream
    (the sequencer executes them in order, so semantics are preserved)."""
    m = json.loads(bir_bytes)
    _dedupe_ldweights(m)
    n_split = 0

    def fix_block(b):
        nonlocal n_split
        out = []
        for ins in b.get("instructions", []):
            si = ins.get("sync_info")
            waits = (si or {}).get("on_wait", [])
            if len(waits) > 1 and ins.get("engine", "Unassigned") != "Unassigned":
                for w in waits[:-1]:
                    n_split += 1
                    out.append({
                        "debug": ins.get("debug", 0),
                        "engine": ins["engine"],
                        "ins": [],
                        "name": f"{ins['name']}-wsplit{n_split}",
                        "opcode": "NoOp",
                        "outs": [],
                        "sync_info": {"on_update": [], "on_wait": [w]},
                    })
                si["on_wait"] = [waits[-1]]
            out.append(ins)
        b["instructions"] = out
        for ch in b.get("blocks", []):
            fix_block(ch)

    for fn in m["functions"]:
        for b in fn.get("blocks", []):
            fix_block(b)
    return json.dumps(m).encode()


def _get_module():
    global _CACHED_NC
    if _CACHED_NC is None:
        nc = _build_module()
        orig = nc.to_json_bytes
        nc.to_json_bytes = lambda: _legalize_single_wait(orig())
        _CACHED_NC = nc
    return _CACHED_NC


def _host_basis(input_t: np.ndarray, model_stage) -> np.ndarray:
    """Packed stationary weights [120, 32] fp16: col 8g+jj = output jj of
    point-group g (jj 0-3 poly batch, 4-7 fourier batch)."""
    stage = int(model_stage)
    curr = min(stage, 3) if stage >= 0 else 3
    mask = np.zeros(KH, dtype=np.float64)
    for s, e, req in ((0, 3, 1), (3, 9, 2), (9, KH, 3)):
        if curr >= req:
            mask[s:e] = 1.0

    t = np.asarray(input_t, dtype=np.float64)
    Vp = np.stack([t ** i for i in range(4)], axis=0)           # [4, B]
    w = 2.0 * np.pi * np.arange(1, KH + 1, dtype=np.float64)    # [18]
    Cc = np.cos(np.outer(w, t)) * mask[:, None]                 # [18, B]
    Ss = np.sin(np.outer(w, t)) * mask[:, None]                 # [18, B]

    B8 = np.zeros((NCOEF, 8), dtype=np.float64)
    B8[0:4, 0:4] = Vp
    B8[4:22, 4:8] = Cc
    B8[22:40, 4:8] = Ss

    basis = np.zeros((GROUPS * NCOEF, 32), dtype=np.float64)
    for g in range(GROUPS):
        basis[NCOEF * g : NCOEF * (g + 1), 8 * g : 8 * g + 8] = B8
    return basis.astype(np.float16)


def kernel(input_t, poly_coeffs, fourier_a, fourier_b, model_stage):
    global LAST_RESULTS
    input_t = np.asarray(input_t, dtype=np.float32)
    poly_coeffs = np.asarray(poly_coeffs, dtype=np.float32)
    fourier_a = np.asarray(fourier_a, dtype=np.float32)
    fourier_b = np.asarray(fourier_b, dtype=np.float32)
    assert input_t.shape == (B,)
    assert poly_coeffs.shape == (N_POINTS, 4)
    assert fourier_a.shape == (N_POINTS, KH)
    assert fourier_b.shape == (N_POINTS, KH)

    basis = _host_basis(input_t, model_stage)

    # Pack per-core tables [120, C] fp16 with the device's column order:
    # table col (span*U + u)*BANK_COLS + j*MM_N + f  holds point
    # g*C + span*U*BANK_COLS + j*U*MM_N + u*MM_N + f   (j <-> u swapped so
    # each output row covers a contiguous DRAM run).
    W = np.concatenate([poly_coeffs, fourier_a, fourier_b], axis=1)
    W = W.astype(np.float16)                                    # [N, 40]
    Wp = np.zeros((N_CORES, NP, NCOEF), dtype=np.float16)
    Wp[:, :NC] = W.reshape(N_CORES, NC, NCOEF)
    Wp = Wp.reshape(N_CORES, GROUPS, SPANS, JT, U, MM_N, NCOEF)
    Wp = Wp.transpose(0, 1, 6, 2, 4, 3, 5)   # core, g, k, span, u, j, f
    tables = np.ascontiguousarray(Wp).reshape(N_CORES, GROUPS * NCOEF, C)

    nc = _get_module()
    in_maps = [{"table": tables[c], "basis": basis} for c in range(N_CORES)]
    LAST_RESULTS = run_bass_kernel_spmd(nc, in_maps, core_ids=list(range(N_CORES)))
    results = LAST_RESULTS.results

    out = np.concatenate(
        [r["out8"][:, :NC].astype(np.float32) for r in results], axis=1
    )
    return out[0:4], out[4:8]
